# revision 1
# baseline (speedup 1.0000x reference)
"""ComplexMultiHeadAttention on 8 TRN2 NeuronCores (Bass/Tile).

Problem: B=4, S=1024, D_MODEL=1024, N_HEADS=16, D_HEAD=64, complex-valued
activations stored as a trailing dim of size 2 (real, imag).

    q = to_heads(complex_linear(queries, wq));  k, v likewise
    s_r + i*s_i = (q_r + i q_i)(k_r + i k_i)^T / sqrt(dh)
    a_r = softmax(s_r), a_i = softmax(s_i)      (independent softmaxes)
    o = complex_bmm(a, v);  out = complex_linear(concat_heads(o), wo)

Sharding: head-parallel. Core c owns heads {2c, 2c+1} = 128 contiguous dims
of the hidden axis. Each core computes Q/K/V projections for its 128 output
dims (weights row-sliced), runs attention for its 8 (batch, head) pairs, and
computes a partial O-projection (wo column-sliced on its 128 input dims)
over all 1024 output dims. The host sums the 8 partial outputs — no
on-device collectives.

Layout: tokens always on the FREE dim, features/keys on partitions, so
every matmul is a natural lhsT.T @ rhs with K=128 contraction:
  - inputs passed transposed: x^T [1024 d, 4096 t]
  - projections produce psum [128 outdims, 512 t]; the complex parts are
    handled by accumulating with sign-folded weight copies (w_i, -w_i).
  - scores are computed TRANSPOSED (s^T [k, q]) from Qcat = [q_r; q_i],
    Kcat_r = [k_r; -k_i], Kcat_i = [k_i; k_r] (all [128, S]) — one K=128
    matmul per 128-key chunk, no accumulation.
  - softmax over k (= partitions) skips max-subtraction (scores are O(1)
    by construction, exp cannot overflow) and takes its partition sums
    with a ones[128,128] f32r matmul that replicates Z across partitions,
    making the 1/Z scale an aligned tensor_mul.
  - V is PE-transposed into token-major packs VA=[v_r|v_i], VB=[-v_i|v_r],
    so attn@V accumulates o_pack [o_r|o_i, q] in a single psum group.
All matmuls run in float32r (TF32-like, 1 cycle/row at N=512 — ~4x the
fp32 rate, ~1.5e-4 relative error). fp32r constraint (probed on HW): the
stationary operand must be full M=128; 128-aligned slices are fine.
PSUM accumulates in f32; outputs are f32.
"""

import os
import numpy as np
import ml_dtypes
from contextlib import ExitStack

import concourse.bass as bass
import concourse.tile as tile
from concourse import bacc, mybir

F32 = mybir.dt.float32
F32R = mybir.dt.float32r
BF16 = mybir.dt.bfloat16
EXP = mybir.ActivationFunctionType.Exp

B, S, D, H, DH = 4, 1024, 1024, 16, 64
NCORES = 8
P = 128            # partitions / chunk size
TBLK = 512         # token block (matmul free dim)
DC = D // P        # 8 d-chunks
KC = S // P        # 8 key chunks per batch
HPC = H // NCORES  # 2 heads per core

_CACHE = {}


def _build():
    nc = bacc.Bacc("TRN2", target_bir_lowering=False, debug=False,
                   num_devices=NCORES)

    NT = (B * S) // TBLK
    x_ap = {}
    for t in ("q", "k", "v"):
        for part in ("r", "i"):
            # tiled-contiguous layout: row block (dc*NT + gt)*P : +P is one
            # [128, 512] tile stored contiguously (single-descriptor DMA)
            x_ap[t + part] = nc.dram_tensor(
                f"x{t}_{part}", [DC * NT * P, TBLK],
                BF16 if t == "v" else F32R, kind="ExternalInput").ap()
    # all projections use per-head combined weights: one psum directly
    # produces the attention layout ([q_r;q_i], [k_r;-k_i], [v_r;v_i])
    w_ap = {}
    for t in ("q", "k", "v"):
        for h in range(HPC):
            for suf in ("a", "b"):
                w_ap[f"{t}{suf}{h}"] = nc.dram_tensor(
                    f"w{t}_{suf}{h}", [P, D],
                    BF16 if t == "v" else F32R, kind="ExternalInput").ap()
    wo_ap = {}
    for suf in ("r", "i", "in"):
        wo_ap[suf] = nc.dram_tensor(
            f"wo_{suf}", [P, D], BF16, kind="ExternalInput").ap()
    ident_ap = nc.dram_tensor("ident", [P, P], BF16, kind="ExternalInput").ap()
    ones_ap = nc.dram_tensor("onesin", [P, P], F32R, kind="ExternalInput").ap()
    # same tiled-contiguous trick for outputs: row block (gt*DC + mc)*P
    po_r = nc.dram_tensor("po_r", [NT * DC * P, TBLK], F32,
                          kind="ExternalOutput").ap()
    po_i = nc.dram_tensor("po_i", [NT * DC * P, TBLK], F32,
                          kind="ExternalOutput").ap()

    with tile.TileContext(nc) as tc, ExitStack() as ctx:
        wpool = ctx.enter_context(tc.tile_pool(name="w", bufs=1))
        xpool = ctx.enter_context(tc.tile_pool(name="x", bufs=12))
        qkpool = ctx.enter_context(tc.tile_pool(name="qk", bufs=2))
        vpool = ctx.enter_context(tc.tile_pool(name="v", bufs=2))
        opool = ctx.enter_context(tc.tile_pool(name="ost", bufs=2))
        upool = ctx.enter_context(tc.tile_pool(name="u", bufs=6))
        zpool = ctx.enter_context(tc.tile_pool(name="z", bufs=2))
        tmppool = ctx.enter_context(tc.tile_pool(name="tmp", bufs=4))
        popool = ctx.enter_context(tc.tile_pool(name="po", bufs=4))
        vstpool = ctx.enter_context(tc.tile_pool(name="vst", bufs=2))
        # PSUM: 8 banks total. projps doubles as the V-transpose target;
        # sps doubles as the O-projection accumulator (same tag).
        projps = ctx.enter_context(tc.tile_pool(name="pp", bufs=2, space="PSUM"))
        sps = ctx.enter_context(tc.tile_pool(name="sp", bufs=2, space="PSUM"))
        zps_pool = ctx.enter_context(tc.tile_pool(name="zp", bufs=1, space="PSUM"))
        ops_pool = ctx.enter_context(tc.tile_pool(name="op", bufs=1, space="PSUM"))

        wt = {}
        for key, ap in list(w_ap.items()):
            wdt = BF16 if key.startswith("v") else F32R
            wt[key] = wpool.tile([P, D], wdt, tag=f"w_{key}", name=f"w_{key}")
            nc.sync.dma_start(wt[key][:], ap[:])
        wot = {}
        for suf, ap in wo_ap.items():
            wot[suf] = wpool.tile([P, D], BF16, tag=f"wo_{suf}",
                                  name=f"wo_{suf}")
            nc.sync.dma_start(wot[suf][:], ap[:])
        ident = wpool.tile([P, P], BF16, tag="ident", name="ident")
        nc.sync.dma_start(ident[:], ident_ap[:])
        ones = wpool.tile([P, P], F32R, tag="ones", name="ones")
        nc.sync.dma_start(ones[:], ones_ap[:])

        for b in range(B):
            qcat = [qkpool.tile([P, S], F32R, tag=f"qcat{h}", name=f"qcat{h}")
                    for h in range(HPC)]
            kcr = [qkpool.tile([P, S], F32R, tag=f"kcr{h}", name=f"kcr{h}")
                   for h in range(HPC)]
            kci = [qkpool.tile([P, S], F32R, tag=f"kci{h}", name=f"kci{h}")
                   for h in range(HPC)]
            va = [vpool.tile([P, S], F32R, tag=f"va{h}", name=f"va{h}")
                  for h in range(HPC)]
            vb = [vpool.tile([P, S], F32R, tag=f"vb{h}", name=f"vb{h}")
                  for h in range(HPC)]
            o_stage = {p: opool.tile([P, S], BF16, tag=f"ost{p}",
                                     name=f"ost{p}")
                       for p in ("r", "i")}

            # ---- projections (per token half-block of 512) ----
            NTv = (B * S) // TBLK
            for t in ("q", "k", "v"):
                xdt = BF16 if t == "v" else F32R
                wA = (wt[t + "a0"], wt[t + "a1"])
                wB = (wt[t + "b0"], wt[t + "b1"])
                for half in range(2):
                    gt = 2 * b + half
                    psr = projps.tile([P, TBLK], F32, tag="projps",
                                      name="projps")
                    psi = projps.tile([P, TBLK], F32, tag="projps",
                                      name="projps")
                    for dc in range(DC):
                        ws = slice(dc * P, (dc + 1) * P)
                        r0 = (dc * NTv + gt) * P
                        xrt = xpool.tile([P, TBLK], xdt, tag="xt", name="xt")
                        nc.sync.dma_start(
                            xrt[:], x_ap[t + "r"][r0:r0 + P, :])
                        nc.tensor.matmul(psr[:], wA[0][:, ws], xrt[:],
                                         start=(dc == 0), stop=False)
                        nc.tensor.matmul(psi[:], wA[1][:, ws], xrt[:],
                                         start=(dc == 0), stop=False)
                    for dc in range(DC):
                        ws = slice(dc * P, (dc + 1) * P)
                        r0 = (dc * NTv + gt) * P
                        xit = xpool.tile([P, TBLK], xdt, tag="xt", name="xt")
                        nc.sync.dma_start(
                            xit[:], x_ap[t + "i"][r0:r0 + P, :])
                        nc.tensor.matmul(psr[:], wB[0][:, ws], xit[:],
                                         start=False, stop=(dc == DC - 1))
                        nc.tensor.matmul(psi[:], wB[1][:, ws], xit[:],
                                         start=False, stop=(dc == DC - 1))
                    hs = slice(half * TBLK, (half + 1) * TBLK)
                    if t == "q":
                        # psX = [q_r(h); q_i(h)] = Qcat directly
                        for h, psx in ((0, psr), (1, psi)):
                            nc.vector.tensor_copy(qcat[h][:, hs], psx[:])
                    elif t == "k":
                        # psX = [k_r(h); -k_i(h)] = Kcat_r directly;
                        # Kcat_i = [k_i; k_r] via one negate + one copy
                        for h, psx in ((0, psr), (1, psi)):
                            nc.vector.tensor_copy(kcr[h][:, hs], psx[:])
                            nc.vector.tensor_scalar_mul(kci[h][0:DH, hs],
                                                        psx[DH:P, :], -1.0)
                            nc.vector.tensor_copy(kci[h][DH:P, hs],
                                                  psx[0:DH, :])
                    else:
                        # psr = [v_r(h0); v_i(h0)], psi = [v_r(h1); v_i(h1)]
                        for h, psx in ((0, psr), (1, psi)):
                            vst = vstpool.tile([P, TBLK], BF16, tag="vst",
                                               name="vst")
                            nc.vector.tensor_copy(vst[:], psx[:])
                            ptb = sps.tile([P, TBLK], BF16, tag="sps",
                                           name="ptb")
                            for blk in range(4):
                                bs = slice(blk * P, (blk + 1) * P)
                                nc.tensor.transpose(ptb[:, bs], vst[:, bs],
                                                    ident[:])
                            # ptb cols per blk: [v_r(h) 64 | v_i(h) 64]
                            base = half * TBLK
                            nc.vector.tensor_copy(
                                va[h][:, base:base + TBLK], ptb[:])
                            vbv = vb[h][:, base:base + TBLK].rearrange(
                                "p (k c) -> p k c", c=P)
                            ptv = ptb[:].rearrange("p (k c) -> p k c", c=P)
                            nc.vector.tensor_copy(vbv[:, :, 0:DH],
                                                  ptv[:, :, DH:P])
                            nc.vector.tensor_copy(vbv[:, :, DH:P],
                                                  ptv[:, :, 0:DH])

            # ---- attention for this batch's 2 heads ----
            for h in range(HPC):
                for qb in range(2):
                    qs = slice(qb * TBLK, (qb + 1) * TBLK)
                    zr = zps_pool.tile([P, TBLK], F32, tag="zr", name="zr")
                    zi = zps_pool.tile([P, TBLK], F32, tag="zi", name="zi")
                    ota = ops_pool.tile([P, TBLK], F32, tag="ota", name="ota")
                    otb = ops_pool.tile([P, TBLK], F32, tag="otb", name="otb")
                    for kc in range(KC):
                        ks = slice(kc * P, (kc + 1) * P)
                        first, last = kc == 0, kc == KC - 1
                        str_ = sps.tile([P, TBLK], F32, tag="sps", name="sps")
                        nc.tensor.matmul(str_[:], kcr[h][:, ks],
                                         qcat[h][:, qs], start=True, stop=True)
                        ur = upool.tile([P, TBLK], F32R, tag="u", name="u")
                        nc.scalar.activation(ur[:], str_[:], EXP)
                        sti = sps.tile([P, TBLK], F32, tag="sps", name="sps")
                        nc.tensor.matmul(sti[:], kci[h][:, ks],
                                         qcat[h][:, qs], start=True, stop=True)
                        ui = upool.tile([P, TBLK], F32R, tag="u", name="u")
                        nc.scalar.activation(ui[:], sti[:], EXP)
                        nc.tensor.matmul(zr[:], ones[:], ur[:],
                                         start=first, stop=last)
                        nc.tensor.matmul(zi[:], ones[:], ui[:],
                                         start=first, stop=last)
                        nc.tensor.matmul(ota[:], va[h][:, ks], ur[:],
                                         start=first, stop=last)
                        nc.tensor.matmul(otb[:], vb[h][:, ks], ui[:],
                                         start=first, stop=last)
                    # o_r = (v_r.T u_r)/Z_r - (v_i.T u_i)/Z_i : each AV term
                    # gets its OWN softmax denominator (independent softmaxes)
                    zinv_r = zpool.tile([P, TBLK], F32, tag="zinv", name="zi_r")
                    nc.vector.reciprocal_approx_fast(zinv_r[:], zr[:])
                    zinv_i = zpool.tile([P, TBLK], F32, tag="zinv", name="zi_i")
                    nc.vector.reciprocal_approx_fast(zinv_i[:], zi[:])
                    tmpa = tmppool.tile([P, TBLK], F32, tag="tmp", name="tmpa")
                    nc.vector.tensor_mul(tmpa[:], ota[:], zinv_r[:])
                    tmpb = tmppool.tile([P, TBLK], F32, tag="tmp", name="tmpb")
                    nc.vector.tensor_mul(tmpb[:], otb[:], zinv_i[:])
                    dst = slice(DH * h, DH * (h + 1))
                    nc.vector.tensor_sub(o_stage["r"][dst, qs], tmpa[0:DH, :],
                                         tmpb[0:DH, :])
                    nc.vector.tensor_add(o_stage["i"][dst, qs], tmpa[DH:P, :],
                                         tmpb[DH:P, :])

            # ---- partial O-projection for this batch ----
            for half in range(2):
                hs = slice(half * TBLK, (half + 1) * TBLK)
                gt = 2 * b + half
                for mc in range(DC):
                    ms = slice(mc * P, (mc + 1) * P)
                    orow = (gt * DC + mc) * P
                    gcols = slice(None)
                    pr = sps.tile([P, TBLK], F32, tag="sps", name="ojpr")
                    nc.tensor.matmul(pr[:], wot["r"][:, ms],
                                     o_stage["r"][:, hs],
                                     start=True, stop=False)
                    nc.tensor.matmul(pr[:], wot["in"][:, ms],
                                     o_stage["i"][:, hs],
                                     start=False, stop=True)
                    sbr = popool.tile([P, TBLK], F32, tag="po", name="po")
                    nc.any.tensor_copy(sbr[:], pr[:])
                    nc.sync.dma_start(po_r[orow:orow + P, :], sbr[:])
                    pi = sps.tile([P, TBLK], F32, tag="sps", name="ojpi")
                    nc.tensor.matmul(pi[:], wot["i"][:, ms],
                                     o_stage["r"][:, hs],
                                     start=True, stop=False)
                    nc.tensor.matmul(pi[:], wot["r"][:, ms],
                                     o_stage["i"][:, hs],
                                     start=False, stop=True)
                    sbi = popool.tile([P, TBLK], F32, tag="po", name="po")
                    nc.any.tensor_copy(sbi[:], pi[:])
                    nc.sync.dma_start(po_i[orow:orow + P, :], sbi[:])

    nc.compile()
    return nc


def _w_sbuf_layout(w_t):
    """[D, 128] weight-transpose slice -> SBUF layout [128, dc*128+o]."""
    return np.ascontiguousarray(
        w_t.reshape(DC, P, P).transpose(1, 0, 2).reshape(P, D))


def _tile_x(xT, dtype):
    """[D, B*S] -> tiled-contiguous [DC*NT*P, TBLK] (rows: (dc*NT+gt)*P)."""
    NT = (B * S) // TBLK
    t = xT.reshape(DC, P, NT, TBLK).transpose(0, 2, 1, 3)
    return np.ascontiguousarray(t.reshape(DC * NT * P, TBLK)).astype(dtype)


def _prepare_in_maps(inputs):
    bf = ml_dtypes.bfloat16
    xs = {}
    for name, t in (("queries", "q"), ("keys", "k"), ("values", "v")):
        x = np.asarray(inputs[name], dtype=np.float32)  # [B,S,D,2]
        flat = x.reshape(B * S, D, 2)
        dt_ = bf if t == "v" else np.float32
        xs[t + "r"] = _tile_x(flat[:, :, 0].T, dt_)
        xs[t + "i"] = _tile_x(flat[:, :, 1].T, dt_)

    scale = np.float32(1.0 / np.sqrt(DH))
    in_maps = []
    for c in range(NCORES):
        rows = slice(P * c, P * (c + 1))
        m = {}
        for t in ("q", "k", "v"):
            for part in ("r", "i"):
                m[f"x{t}_{part}"] = xs[t + part]
        for t, wr_name, wi_name in (("q", "wq_r", "wq_i"),
                                    ("k", "wk_r", "wk_i"),
                                    ("v", "wv_r", "wv_i")):
            s = scale if t == "q" else np.float32(1.0)
            wdt = bf if t == "v" else np.float32
            wr = np.asarray(inputs[wr_name], dtype=np.float32)[rows] * s
            wi = np.asarray(inputs[wi_name], dtype=np.float32)[rows] * s
            for h in range(HPC):
                hr = slice(DH * h, DH * (h + 1))
                if t == "q":
                    wa = np.concatenate([wr[hr].T, wi[hr].T], axis=1)
                    wb = np.concatenate([-wi[hr].T, wr[hr].T], axis=1)
                elif t == "k":
                    wa = np.concatenate([wr[hr].T, -wi[hr].T], axis=1)
                    wb = np.concatenate([-wi[hr].T, -wr[hr].T], axis=1)
                else:
                    wa = np.concatenate([wr[hr].T, wi[hr].T], axis=1)
                    wb = np.concatenate([-wi[hr].T, wr[hr].T], axis=1)
                m[f"w{t}_a{h}"] = _w_sbuf_layout(wa).astype(wdt)
                m[f"w{t}_b{h}"] = _w_sbuf_layout(wb).astype(wdt)
        wo_r = np.asarray(inputs["wo_r"], dtype=np.float32)[:, rows]  # [D,128]
        wo_i = np.asarray(inputs["wo_i"], dtype=np.float32)[:, rows]
        m["wo_r"] = np.ascontiguousarray(wo_r.T).astype(bf)  # [128 d, 1024 m]
        m["wo_i"] = np.ascontiguousarray(wo_i.T).astype(bf)
        m["wo_in"] = np.ascontiguousarray(-wo_i.T).astype(bf)
        m["ident"] = np.eye(P, dtype=bf)
        m["onesin"] = np.ones((P, P), dtype=np.float32)
        in_maps.append(m)
    return in_maps


LAST_RESULT = None


def _run(inputs, trace=False):
    global LAST_RESULT
    from concourse.bass_utils import run_bass_kernel_spmd
    if "nc" not in _CACHE:
        _CACHE["nc"] = _build()
    nc = _CACHE["nc"]
    in_maps = _prepare_in_maps(inputs)
    if trace:
        os.environ.pop("BASS_NEVER_TRACE", None)
    else:
        os.environ["BASS_NEVER_TRACE"] = "1"
    res = run_bass_kernel_spmd(nc, in_maps, core_ids=list(range(NCORES)),
                               trace=trace)
    LAST_RESULT = res
    NT = (B * S) // TBLK
    acc_r = np.zeros((NT * DC * P, TBLK), np.float32)
    acc_i = np.zeros((NT * DC * P, TBLK), np.float32)
    for c in range(NCORES):
        acc_r += res.results[c]["po_r"]
        acc_i += res.results[c]["po_i"]

    def untile(po):
        # [NT*DC*P, TBLK] rows (gt*DC+mc)*P -> [D, B*S] -> [B,S,D]
        t = po.reshape(NT, DC, P, TBLK).transpose(1, 2, 0, 3)
        return np.ascontiguousarray(t.reshape(D, B * S)).T.reshape(B, S, D)

    out = np.empty((B, S, D, 2), np.float32)
    out[..., 0] = untile(acc_r)
    out[..., 1] = untile(acc_i)
    return out


def kernel(**inputs):
    return _run(inputs, trace=False)



# revision 2
# speedup vs baseline: 1.0785x; 1.0785x over previous
"""ComplexMultiHeadAttention on 8 TRN2 NeuronCores (Bass/Tile).

Problem: B=4, S=1024, D_MODEL=1024, N_HEADS=16, D_HEAD=64, complex-valued
activations stored as a trailing dim of size 2 (real, imag).

    q = to_heads(complex_linear(queries, wq));  k, v likewise
    s_r + i*s_i = (q_r + i q_i)(k_r + i k_i)^T / sqrt(dh)
    a_r = softmax(s_r), a_i = softmax(s_i)      (independent softmaxes)
    o = complex_bmm(a, v);  out = complex_linear(concat_heads(o), wo)

Sharding: head-parallel. Core c owns heads {2c, 2c+1} = 128 contiguous dims
of the hidden axis. Each core computes Q/K/V projections for its 128 output
dims (weights row-sliced), runs attention for its 8 (batch, head) pairs, and
computes a partial O-projection (wo column-sliced on its 128 input dims)
over all 1024 output dims. The host sums the 8 partial outputs — no
on-device collectives.

Layout: tokens always on the FREE dim, features/keys on partitions, so
every matmul is a natural lhsT.T @ rhs with K=128 contraction:
  - inputs passed transposed: x^T [1024 d, 4096 t]
  - projections produce psum [128 outdims, 512 t]; the complex parts are
    handled by accumulating with sign-folded weight copies (w_i, -w_i).
  - scores are computed TRANSPOSED (s^T [k, q]) from Qcat = [q_r; q_i],
    Kcat_r = [k_r; -k_i], Kcat_i = [k_i; k_r] (all [128, S]) — one K=128
    matmul per 128-key chunk, no accumulation.  The r and i score chunks
    land in the two banks of one wide [128, 1024] PSUM tile so a single
    wide EXP activation covers both (amortizes the scalar engine's
    per-instruction overhead).
  - softmax over k (= partitions) skips max-subtraction (scores are O(1)
    by construction, exp cannot overflow) and takes its partition sums
    with a ones[128,128] matmul that replicates Z across partitions,
    making the 1/Z scale an aligned tensor_mul.
  - V is PE-transposed into token-major packs VA=[v_r|v_i], VB=[-v_i|v_r],
    so attn@V accumulates o_pack [o_r|o_i, q] in a single psum group.
All matmuls run in BF16 (1 column/cycle on the PE vs 1.5 for f32r) with
f32 PSUM accumulation.  The attention inner loop is software-pipelined:
scores for chunk kc+1 are emitted before the Z/AV consumers of chunk kc,
so the PE never stalls waiting for the scalar engine's exp.
PSUM budget (8 banks): wide score/proj pool 2x2 + wide Z 2 + wide AV 2.
"""

import os
import numpy as np
import ml_dtypes
from contextlib import ExitStack

import concourse.bass as bass
import concourse.tile as tile
from concourse import bacc, mybir

F32 = mybir.dt.float32
BF16 = mybir.dt.bfloat16
EXP = mybir.ActivationFunctionType.Exp

B, S, D, H, DH = 4, 1024, 1024, 16, 64
NCORES = 8
P = 128            # partitions / chunk size
TBLK = 512         # token block (matmul free dim)
WBLK = 2 * TBLK    # wide psum tile (2 banks)
DC = D // P        # 8 d-chunks
KC = S // P        # 8 key chunks per batch
HPC = H // NCORES  # 2 heads per core

_CACHE = {}


def _build():
    nc = bacc.Bacc("TRN2", target_bir_lowering=False, debug=False,
                   num_devices=NCORES)

    NT = (B * S) // TBLK
    x_ap = {}
    for t in ("q", "k", "v"):
        for part in ("r", "i"):
            # tiled-contiguous layout: row block (dc*NT + gt)*P : +P is one
            # [128, 512] tile stored contiguously (single-descriptor DMA)
            x_ap[t + part] = nc.dram_tensor(
                f"x{t}_{part}", [DC * NT * P, TBLK],
                BF16, kind="ExternalInput").ap()
    # all projections use per-head combined weights: one psum directly
    # produces the attention layout ([q_r;q_i], [k_r;-k_i], [v_r;v_i])
    w_ap = {}
    for t in ("q", "k", "v"):
        for h in range(HPC):
            for suf in ("a", "b"):
                w_ap[f"{t}{suf}{h}"] = nc.dram_tensor(
                    f"w{t}_{suf}{h}", [P, D], BF16, kind="ExternalInput").ap()
    wo_ap = {}
    for suf in ("r", "i", "in"):
        wo_ap[suf] = nc.dram_tensor(
            f"wo_{suf}", [P, D], BF16, kind="ExternalInput").ap()
    ident_ap = nc.dram_tensor("ident", [P, P], F32, kind="ExternalInput").ap()
    ones_ap = nc.dram_tensor("onesin", [P, P], BF16, kind="ExternalInput").ap()
    # same tiled-contiguous trick for outputs: row block (gt*DC + mc)*P
    po_r = nc.dram_tensor("po_r", [NT * DC * P, TBLK], BF16,
                          kind="ExternalOutput").ap()
    po_i = nc.dram_tensor("po_i", [NT * DC * P, TBLK], BF16,
                          kind="ExternalOutput").ap()

    with tile.TileContext(nc) as tc, ExitStack() as ctx:
        wpool = ctx.enter_context(tc.tile_pool(name="w", bufs=1))
        xpool = ctx.enter_context(tc.tile_pool(name="x", bufs=16))
        qkpool = ctx.enter_context(tc.tile_pool(name="qk", bufs=2))
        vpool = ctx.enter_context(tc.tile_pool(name="v", bufs=2))
        opool = ctx.enter_context(tc.tile_pool(name="ost", bufs=2))
        upool = ctx.enter_context(tc.tile_pool(name="u", bufs=3))
        zpool = ctx.enter_context(tc.tile_pool(name="z", bufs=2))
        tmppool = ctx.enter_context(tc.tile_pool(name="tmp", bufs=4))
        popool = ctx.enter_context(tc.tile_pool(name="po", bufs=4))
        vstpool = ctx.enter_context(tc.tile_pool(name="vst", bufs=2))
        # PSUM: 8 banks total, all as wide 2-bank [128, 1024] tiles:
        #   sps x2 (proj / scores / o-proj, double-buffered)
        #   zw  x1 (Z_r | Z_i accumulators; V-transpose staging off-phase)
        #   ow  x1 (AV pack-A | pack-B accumulators)
        sps = ctx.enter_context(tc.tile_pool(name="sp", bufs=2, space="PSUM"))
        zps_pool = ctx.enter_context(tc.tile_pool(name="zp", bufs=1,
                                                  space="PSUM"))
        ops_pool = ctx.enter_context(tc.tile_pool(name="op", bufs=1,
                                                  space="PSUM"))

        wt = {}
        for key, ap in list(w_ap.items()):
            wt[key] = wpool.tile([P, D], BF16, tag=f"w_{key}", name=f"w_{key}")
            nc.sync.dma_start(wt[key][:], ap[:])
        wot = {}
        for suf, ap in wo_ap.items():
            wot[suf] = wpool.tile([P, D], BF16, tag=f"wo_{suf}",
                                  name=f"wo_{suf}")
            nc.sync.dma_start(wot[suf][:], ap[:])
        ident = wpool.tile([P, P], F32, tag="ident", name="ident")
        nc.sync.dma_start(ident[:], ident_ap[:])
        ones = wpool.tile([P, P], BF16, tag="ones", name="ones")
        nc.sync.dma_start(ones[:], ones_ap[:])

        for b in range(B):
            qcat = [qkpool.tile([P, S], BF16, tag=f"qcat{h}", name=f"qcat{h}")
                    for h in range(HPC)]
            kcr = [qkpool.tile([P, S], BF16, tag=f"kcr{h}", name=f"kcr{h}")
                   for h in range(HPC)]
            kci = [qkpool.tile([P, S], BF16, tag=f"kci{h}", name=f"kci{h}")
                   for h in range(HPC)]
            va = [vpool.tile([P, S], BF16, tag=f"va{h}", name=f"va{h}")
                  for h in range(HPC)]
            vb = [vpool.tile([P, S], BF16, tag=f"vb{h}", name=f"vb{h}")
                  for h in range(HPC)]
            o_stage = {p: opool.tile([P, S], BF16, tag=f"ost{p}",
                                     name=f"ost{p}")
                       for p in ("r", "i")}

            # ---- projections (per token half-block of 512) ----
            NTv = (B * S) // TBLK
            for t in ("q", "k", "v"):
                wA = (wt[t + "a0"], wt[t + "a1"])
                wB = (wt[t + "b0"], wt[t + "b1"])
                for half in range(2):
                    gt = 2 * b + half
                    # psr|psi = two banks of one wide psum tile
                    pwide = sps.tile([P, WBLK], F32, tag="sps", name="projw")
                    psr = pwide[:, 0:TBLK]
                    psi = pwide[:, TBLK:WBLK]
                    for dc in range(DC):
                        ws = slice(dc * P, (dc + 1) * P)
                        r0 = (dc * NTv + gt) * P
                        xrt = xpool.tile([P, TBLK], BF16, tag="xt", name="xt")
                        nc.sync.dma_start(
                            xrt[:], x_ap[t + "r"][r0:r0 + P, :])
                        nc.tensor.matmul(psr, wA[0][:, ws], xrt[:],
                                         start=(dc == 0), stop=False)
                        nc.tensor.matmul(psi, wA[1][:, ws], xrt[:],
                                         start=(dc == 0), stop=False)
                    for dc in range(DC):
                        ws = slice(dc * P, (dc + 1) * P)
                        r0 = (dc * NTv + gt) * P
                        xit = xpool.tile([P, TBLK], BF16, tag="xt", name="xt")
                        nc.sync.dma_start(
                            xit[:], x_ap[t + "i"][r0:r0 + P, :])
                        nc.tensor.matmul(psr, wB[0][:, ws], xit[:],
                                         start=False, stop=(dc == DC - 1))
                        nc.tensor.matmul(psi, wB[1][:, ws], xit[:],
                                         start=False, stop=(dc == DC - 1))
                    hs = slice(half * TBLK, (half + 1) * TBLK)
                    if t == "q":
                        # psX = [q_r(h); q_i(h)] = Qcat directly
                        for h, psx in ((0, psr), (1, psi)):
                            nc.vector.tensor_copy(qcat[h][:, hs], psx)
                    elif t == "k":
                        # psX = [k_r(h); -k_i(h)] = Kcat_r directly;
                        # Kcat_i = [k_i; k_r] via one negate + one copy
                        for h, psx in ((0, psr), (1, psi)):
                            nc.vector.tensor_copy(kcr[h][:, hs], psx)
                            nc.vector.tensor_scalar_mul(
                                kci[h][0:DH, hs], psx[DH:P, :], -1.0)
                            nc.vector.tensor_copy(kci[h][DH:P, hs],
                                                  psx[0:DH, :])
                    else:
                        # psr = [v_r(h0); v_i(h0)], psi = [v_r(h1); v_i(h1)]
                        # transpose through the (off-phase) zw wide tile
                        twide = zps_pool.tile([P, WBLK], F32, tag="zw",
                                              name="vtw")
                        for h, psx in ((0, psr), (1, psi)):
                            vst = vstpool.tile([P, TBLK], F32, tag="vst",
                                               name="vst")
                            nc.vector.tensor_copy(vst[:], psx)
                            ptb = twide[:, h * TBLK:(h + 1) * TBLK]
                            for blk in range(4):
                                bs = slice(blk * P, (blk + 1) * P)
                                nc.tensor.transpose(ptb[:, bs], vst[:, bs],
                                                    ident[:])
                            # ptb cols per blk: [v_r(h) 64 | v_i(h) 64]
                            base = half * TBLK
                            nc.vector.tensor_copy(
                                va[h][:, base:base + TBLK], ptb)
                            vbv = vb[h][:, base:base + TBLK].rearrange(
                                "p (k c) -> p k c", c=P)
                            ptv = ptb.rearrange("p (k c) -> p k c", c=P)
                            nc.vector.tensor_copy(vbv[:, :, 0:DH],
                                                  ptv[:, :, DH:P])
                            nc.vector.tensor_copy(vbv[:, :, DH:P],
                                                  ptv[:, :, 0:DH])

            # ---- attention for this batch's 2 heads ----
            # flattened + software-pipelined: scores for unit n+1 are
            # emitted before the Z/AV consumers of unit n, so the PE keeps
            # streaming while the scalar engine runs exp(n).
            units = [(h, qb, kc)
                     for h in range(HPC) for qb in range(2)
                     for kc in range(KC)]
            swides = [None] * len(units)
            uwides = [None] * len(units)
            accs = {}

            def emit_scores(n):
                h, qb, kc = units[n]
                qs = slice(qb * TBLK, (qb + 1) * TBLK)
                ks = slice(kc * P, (kc + 1) * P)
                sw = sps.tile([P, WBLK], F32, tag="sps", name="scorew")
                nc.tensor.matmul(sw[:, 0:TBLK], kcr[h][:, ks],
                                 qcat[h][:, qs], start=True, stop=True)
                nc.tensor.matmul(sw[:, TBLK:WBLK], kci[h][:, ks],
                                 qcat[h][:, qs], start=True, stop=True)
                swides[n] = sw

            emit_scores(0)
            for n, (h, qb, kc) in enumerate(units):
                if n + 1 < len(units):
                    emit_scores(n + 1)
                # single wide exp over [s_r | s_i]
                uw = upool.tile([P, WBLK], BF16, tag="u", name="u")
                nc.scalar.activation(uw[:], swides[n][:], EXP)
                uwides[n] = uw
                ur = uw[:, 0:TBLK]
                ui = uw[:, TBLK:WBLK]
                ks = slice(kc * P, (kc + 1) * P)
                first, last = kc == 0, kc == KC - 1
                if first:
                    zw = zps_pool.tile([P, WBLK], F32, tag="zw", name="zw")
                    ow = ops_pool.tile([P, WBLK], F32, tag="ow", name="ow")
                    accs[(h, qb)] = (zw, ow)
                else:
                    zw, ow = accs[(h, qb)]
                nc.tensor.matmul(zw[:, 0:TBLK], ones[:], ur,
                                 start=first, stop=last)
                nc.tensor.matmul(zw[:, TBLK:WBLK], ones[:], ui,
                                 start=first, stop=last)
                nc.tensor.matmul(ow[:, 0:TBLK], va[h][:, ks], ur,
                                 start=first, stop=last)
                nc.tensor.matmul(ow[:, TBLK:WBLK], vb[h][:, ks], ui,
                                 start=first, stop=last)
                if last:
                    # o_r = (v_r.T u_r)/Z_r - (v_i.T u_i)/Z_i : each AV term
                    # gets its OWN softmax denominator
                    qs = slice(qb * TBLK, (qb + 1) * TBLK)
                    zinv_r = zpool.tile([P, TBLK], F32, tag="zinv",
                                        name="zi_r")
                    nc.vector.reciprocal_approx_fast(zinv_r[:],
                                                     zw[:, 0:TBLK])
                    zinv_i = zpool.tile([P, TBLK], F32, tag="zinv",
                                        name="zi_i")
                    nc.vector.reciprocal_approx_fast(zinv_i[:],
                                                     zw[:, TBLK:WBLK])
                    tmpa = tmppool.tile([P, TBLK], F32, tag="tmp",
                                        name="tmpa")
                    nc.vector.tensor_mul(tmpa[:], ow[:, 0:TBLK], zinv_r[:])
                    tmpb = tmppool.tile([P, TBLK], F32, tag="tmp",
                                        name="tmpb")
                    nc.vector.tensor_mul(tmpb[:], ow[:, TBLK:WBLK],
                                         zinv_i[:])
                    dst = slice(DH * h, DH * (h + 1))
                    nc.vector.tensor_sub(o_stage["r"][dst, qs],
                                         tmpa[0:DH, :], tmpb[0:DH, :])
                    nc.vector.tensor_add(o_stage["i"][dst, qs],
                                         tmpa[DH:P, :], tmpb[DH:P, :])

            # ---- partial O-projection for this batch ----
            for half in range(2):
                hs = slice(half * TBLK, (half + 1) * TBLK)
                gt = 2 * b + half
                for mc in range(DC):
                    ms = slice(mc * P, (mc + 1) * P)
                    orow = (gt * DC + mc) * P
                    ow2 = sps.tile([P, WBLK], F32, tag="sps", name="ojw")
                    pr = ow2[:, 0:TBLK]
                    pi = ow2[:, TBLK:WBLK]
                    nc.tensor.matmul(pr, wot["r"][:, ms],
                                     o_stage["r"][:, hs],
                                     start=True, stop=False)
                    nc.tensor.matmul(pr, wot["in"][:, ms],
                                     o_stage["i"][:, hs],
                                     start=False, stop=True)
                    nc.tensor.matmul(pi, wot["i"][:, ms],
                                     o_stage["r"][:, hs],
                                     start=True, stop=False)
                    nc.tensor.matmul(pi, wot["r"][:, ms],
                                     o_stage["i"][:, hs],
                                     start=False, stop=True)
                    sbr = popool.tile([P, TBLK], BF16, tag="po", name="po")
                    nc.vector.tensor_copy(sbr[:], pr)
                    nc.sync.dma_start(po_r[orow:orow + P, :], sbr[:])
                    sbi = popool.tile([P, TBLK], BF16, tag="po", name="po")
                    nc.vector.tensor_copy(sbi[:], pi)
                    nc.sync.dma_start(po_i[orow:orow + P, :], sbi[:])

    nc.compile()
    return nc


def _w_sbuf_layout(w_t):
    """[D, 128] weight-transpose slice -> SBUF layout [128, dc*128+o]."""
    return np.ascontiguousarray(
        w_t.reshape(DC, P, P).transpose(1, 0, 2).reshape(P, D))


def _tile_x(xT, dtype):
    """[D, B*S] -> tiled-contiguous [DC*NT*P, TBLK] (rows: (dc*NT+gt)*P)."""
    NT = (B * S) // TBLK
    t = xT.reshape(DC, P, NT, TBLK).transpose(0, 2, 1, 3)
    return np.ascontiguousarray(t.reshape(DC * NT * P, TBLK)).astype(dtype)


def _prepare_in_maps(inputs):
    bf = ml_dtypes.bfloat16
    xs = {}
    for name, t in (("queries", "q"), ("keys", "k"), ("values", "v")):
        x = np.asarray(inputs[name], dtype=np.float32)  # [B,S,D,2]
        flat = x.reshape(B * S, D, 2)
        xs[t + "r"] = _tile_x(flat[:, :, 0].T, bf)
        xs[t + "i"] = _tile_x(flat[:, :, 1].T, bf)

    scale = np.float32(1.0 / np.sqrt(DH))
    in_maps = []
    for c in range(NCORES):
        rows = slice(P * c, P * (c + 1))
        m = {}
        for t in ("q", "k", "v"):
            for part in ("r", "i"):
                m[f"x{t}_{part}"] = xs[t + part]
        for t, wr_name, wi_name in (("q", "wq_r", "wq_i"),
                                    ("k", "wk_r", "wk_i"),
                                    ("v", "wv_r", "wv_i")):
            s = scale if t == "q" else np.float32(1.0)
            wr = np.asarray(inputs[wr_name], dtype=np.float32)[rows] * s
            wi = np.asarray(inputs[wi_name], dtype=np.float32)[rows] * s
            for h in range(HPC):
                hr = slice(DH * h, DH * (h + 1))
                if t == "q":
                    wa = np.concatenate([wr[hr].T, wi[hr].T], axis=1)
                    wb = np.concatenate([-wi[hr].T, wr[hr].T], axis=1)
                elif t == "k":
                    wa = np.concatenate([wr[hr].T, -wi[hr].T], axis=1)
                    wb = np.concatenate([-wi[hr].T, -wr[hr].T], axis=1)
                else:
                    wa = np.concatenate([wr[hr].T, wi[hr].T], axis=1)
                    wb = np.concatenate([-wi[hr].T, wr[hr].T], axis=1)
                m[f"w{t}_a{h}"] = _w_sbuf_layout(wa).astype(bf)
                m[f"w{t}_b{h}"] = _w_sbuf_layout(wb).astype(bf)
        wo_r = np.asarray(inputs["wo_r"], dtype=np.float32)[:, rows]  # [D,128]
        wo_i = np.asarray(inputs["wo_i"], dtype=np.float32)[:, rows]
        m["wo_r"] = np.ascontiguousarray(wo_r.T).astype(bf)  # [128 d, 1024 m]
        m["wo_i"] = np.ascontiguousarray(wo_i.T).astype(bf)
        m["wo_in"] = np.ascontiguousarray(-wo_i.T).astype(bf)
        m["ident"] = np.eye(P, dtype=np.float32)
        m["onesin"] = np.ones((P, P), dtype=bf)
        in_maps.append(m)
    return in_maps


LAST_RESULT = None


def _run(inputs, trace=False):
    global LAST_RESULT
    from concourse.bass_utils import run_bass_kernel_spmd
    if "nc" not in _CACHE:
        _CACHE["nc"] = _build()
    nc = _CACHE["nc"]
    in_maps = _prepare_in_maps(inputs)
    if trace:
        os.environ.pop("BASS_NEVER_TRACE", None)
    else:
        os.environ["BASS_NEVER_TRACE"] = "1"
    res = run_bass_kernel_spmd(nc, in_maps, core_ids=list(range(NCORES)),
                               trace=trace)
    LAST_RESULT = res
    NT = (B * S) // TBLK
    acc_r = np.zeros((NT * DC * P, TBLK), np.float32)
    acc_i = np.zeros((NT * DC * P, TBLK), np.float32)
    for c in range(NCORES):
        acc_r += res.results[c]["po_r"].astype(np.float32)
        acc_i += res.results[c]["po_i"].astype(np.float32)

    def untile(po):
        # [NT*DC*P, TBLK] rows (gt*DC+mc)*P -> [D, B*S] -> [B,S,D]
        t = po.reshape(NT, DC, P, TBLK).transpose(1, 2, 0, 3)
        return np.ascontiguousarray(t.reshape(D, B * S)).T.reshape(B, S, D)

    out = np.empty((B, S, D, 2), np.float32)
    out[..., 0] = untile(acc_r)
    out[..., 1] = untile(acc_i)
    return out


def kernel(**inputs):
    return _run(inputs, trace=False)


# revision 4
# speedup vs baseline: 1.2252x; 1.1360x over previous
"""ComplexMultiHeadAttention on 8 TRN2 NeuronCores (Bass/Tile).

Problem: B=4, S=1024, D_MODEL=1024, N_HEADS=16, D_HEAD=64, complex-valued
activations stored as a trailing dim of size 2 (real, imag).

    q = to_heads(complex_linear(queries, wq));  k, v likewise
    s_r + i*s_i = (q_r + i q_i)(k_r + i k_i)^T / sqrt(dh)
    a_r = softmax(s_r), a_i = softmax(s_i)      (independent softmaxes)
    o = complex_bmm(a, v);  out = complex_linear(concat_heads(o), wo)

Sharding: head-parallel. Core c owns heads {2c, 2c+1} = 128 contiguous dims
of the hidden axis. Each core computes Q/K/V projections for its 128 output
dims (weights row-sliced), runs attention for its 8 (batch, head) pairs, and
computes a partial O-projection (wo column-sliced on its 128 input dims)
over all 1024 output dims. The host sums the 8 partial outputs — no
on-device collectives.

Performance notes (TRN2):
  - The PE p-state ramps: any idle gap drops the clock to 1.2 GHz for the
    next ~3us. The whole program is therefore emitted as one continuous
    tensor stream: attention(b) -> proj(b+1) -> oproj(b) -> attention(b+1),
    with the attention inner loop software-pipelined (scores for chunk n+1
    are emitted before the exp-gated consumers of chunk n).
  - DMA cost is per-partition-LINE (~5.6ns/line regardless of 1KB vs 2KB),
    so inputs use a partition-major DRAM layout ([gt*128+p, dc*512+tok])
    giving 8KB contiguous lines: one 1MB DMA per (tensor, part, gt) instead
    of eight 128KB DMAs with 1KB lines. Outputs are batched the same way.
  - V is transposed to token-major via DMA-transpose on the scalar HWDGE
    queue (not the PE), with V projected FIRST so the transposes overlap
    the q/k projection matmuls.
  - scores r|i land in the two banks of one wide [128,1024] PSUM tile; a
    single wide EXP covers both (scalar engine issue rate ~1.15us/unit vs
    1.28us of tensor work per unit -> attention stays tensor-paced).
  - All matmuls bf16 (f32 PSUM accumulation); softmax over k (=partitions)
    skips max-subtraction (scores are O(1) by construction) and takes Z
    with a ones[128,128] matmul; 1/Z is one wide reciprocal + one wide mul.
PSUM budget (8 banks): wide proj/score pool 2x2 + wide Z 2 + wide AV 2.
"""

import os
import numpy as np
import ml_dtypes
from contextlib import ExitStack

import concourse.bass as bass
import concourse.tile as tile
from concourse import bacc, mybir

F32 = mybir.dt.float32
BF16 = mybir.dt.bfloat16
EXP = mybir.ActivationFunctionType.Exp

B, S, D, H, DH = 4, 1024, 1024, 16, 64
NCORES = 8
P = 128            # partitions / chunk size
TBLK = 512         # token block (matmul free dim)
WBLK = 2 * TBLK    # wide psum tile (2 banks)
DC = D // P        # 8 d-chunks
KC = S // P        # 8 key chunks per batch
HPC = H // NCORES  # 2 heads per core
NT = (B * S) // TBLK

_CACHE = {}


def _build():
    nc = bacc.Bacc("TRN2", target_bir_lowering=False, debug=False,
                   num_devices=NCORES)

    # partition-major tiled layout: row gt*128+p, col dc*512+tok
    x_ap = {}
    for t in ("q", "k", "v"):
        for part in ("r", "i"):
            x_ap[t + part] = nc.dram_tensor(
                f"x{t}_{part}", [NT * P, DC * TBLK],
                BF16, kind="ExternalInput").ap()
    w_ap = {}
    for t in ("q", "k", "v"):
        for h in range(HPC):
            for suf in ("a", "b"):
                w_ap[f"{t}{suf}{h}"] = nc.dram_tensor(
                    f"w{t}_{suf}{h}", [P, D], BF16, kind="ExternalInput").ap()
    wo_ap = {}
    for suf in ("r", "i", "in"):
        wo_ap[suf] = nc.dram_tensor(
            f"wo_{suf}", [P, D], BF16, kind="ExternalInput").ap()
    ones_ap = nc.dram_tensor("onesin", [P, P], BF16, kind="ExternalInput").ap()
    # output: row gt*128+p, col (2*mc+ri)*512+tok  (r/i interleaved per mc)
    po_ap = nc.dram_tensor("po", [NT * P, 2 * DC * TBLK], BF16,
                           kind="ExternalOutput").ap()

    with tile.TileContext(nc) as tc, ExitStack() as ctx:
        wpool = ctx.enter_context(tc.tile_pool(name="w", bufs=1))
        xpool = ctx.enter_context(tc.tile_pool(name="x", bufs=6))
        qkpool = ctx.enter_context(tc.tile_pool(name="qk", bufs=2))
        vpool = ctx.enter_context(tc.tile_pool(name="v", bufs=2))
        opool = ctx.enter_context(tc.tile_pool(name="ost", bufs=2))
        upool = ctx.enter_context(tc.tile_pool(name="u", bufs=3))
        zpool = ctx.enter_context(tc.tile_pool(name="z", bufs=2))
        tmppool = ctx.enter_context(tc.tile_pool(name="tmp", bufs=2))
        popool = ctx.enter_context(tc.tile_pool(name="po", bufs=2))
        vstpool = ctx.enter_context(tc.tile_pool(name="vst", bufs=4))
        # PSUM: 8 banks as wide 2-bank [128,1024] tiles:
        #   sps x2 (proj accum / scores / o-proj, double-buffered)
        #   zw  x1 (Z_r | Z_i accumulators)
        #   ow  x1 (AV pack-A | pack-B accumulators)
        sps = ctx.enter_context(tc.tile_pool(name="sp", bufs=2, space="PSUM"))
        zps_pool = ctx.enter_context(tc.tile_pool(name="zp", bufs=1,
                                                  space="PSUM"))
        ops_pool = ctx.enter_context(tc.tile_pool(name="op", bufs=1,
                                                  space="PSUM"))

        # weights on the scalar HWDGE queue (idle at start; sync queue
        # starts on the batch-0 x loads in parallel)
        wt = {}
        for key, ap in list(w_ap.items()):
            wt[key] = wpool.tile([P, D], BF16, tag=f"w_{key}", name=f"w_{key}")
            nc.scalar.dma_start(wt[key][:], ap[:])
        wot = {}
        for suf, ap in wo_ap.items():
            wot[suf] = wpool.tile([P, D], BF16, tag=f"wo_{suf}",
                                  name=f"wo_{suf}")
            nc.scalar.dma_start(wot[suf][:], ap[:])
        ones = wpool.tile([P, P], BF16, tag="ones", name="ones")
        nc.scalar.dma_start(ones[:], ones_ap[:])

        xtiles = {}

        def emit_xloads(b):
            # v first (projection order is v,q,k)
            for t in ("v", "q", "k"):
                for part in ("r", "i"):
                    for half in range(2):
                        gt = 2 * b + half
                        xt = xpool.tile([P, DC * TBLK], BF16, tag="xt",
                                        name="xt")
                        nc.sync.dma_start(
                            xt[:], x_ap[t + part][gt * P:(gt + 1) * P, :])
                        xtiles[(b, t, part, half)] = xt

        def emit_proj(b, qcat, kcr, kci, va, vb):
            # v FIRST so its DMA-transposes (scalar queue) overlap the q/k
            # projection matmuls
            for t in ("v", "q", "k"):
                wA = (wt[t + "a0"], wt[t + "a1"])
                wB = (wt[t + "b0"], wt[t + "b1"])
                for half in range(2):
                    xr = xtiles.pop((b, t, "r", half))
                    xi = xtiles.pop((b, t, "i", half))
                    pwide = sps.tile([P, WBLK], F32, tag="sps", name="projw")
                    psr = pwide[:, 0:TBLK]
                    psi = pwide[:, TBLK:WBLK]
                    for dc in range(DC):
                        ws = slice(dc * P, (dc + 1) * P)
                        xs_ = slice(dc * TBLK, (dc + 1) * TBLK)
                        nc.tensor.matmul(psr, wA[0][:, ws], xr[:, xs_],
                                         start=(dc == 0), stop=False)
                        nc.tensor.matmul(psi, wA[1][:, ws], xr[:, xs_],
                                         start=(dc == 0), stop=False)
                    for dc in range(DC):
                        ws = slice(dc * P, (dc + 1) * P)
                        xs_ = slice(dc * TBLK, (dc + 1) * TBLK)
                        nc.tensor.matmul(psr, wB[0][:, ws], xi[:, xs_],
                                         start=False, stop=(dc == DC - 1))
                        nc.tensor.matmul(psi, wB[1][:, ws], xi[:, xs_],
                                         start=False, stop=(dc == DC - 1))
                    hs = slice(half * TBLK, (half + 1) * TBLK)
                    if t == "q":
                        # psX = [q_r(h); q_i(h)] = Qcat directly
                        for h, psx in ((0, psr), (1, psi)):
                            nc.vector.tensor_copy(qcat[h][:, hs], psx)
                    elif t == "k":
                        # psX = [k_r(h); -k_i(h)] = Kcat_r directly;
                        # Kcat_i = [k_i; k_r] via one negate + one copy
                        for h, psx in ((0, psr), (1, psi)):
                            nc.vector.tensor_copy(kcr[h][:, hs], psx)
                            nc.vector.tensor_scalar_mul(
                                kci[h][0:DH, hs], psx[DH:P, :], -1.0)
                            nc.vector.tensor_copy(kci[h][DH:P, hs],
                                                  psx[0:DH, :])
                    else:
                        # psr = [v_r(h0); v_i(h0)], psi = [v_r(h1); v_i(h1)]
                        # DMA-transpose (scalar HWDGE) to token-major packs
                        for h, psx in ((0, psr), (1, psi)):
                            vst = vstpool.tile([P, TBLK], BF16, tag="vst",
                                               name="vst")
                            nc.vector.tensor_copy(vst[:], psx)
                            for blk in range(4):
                                kcg = half * 4 + blk
                                nc.scalar.dma_start(
                                    va[h][:, kcg * P:(kcg + 1) * P],
                                    vst[:, blk * P:(blk + 1) * P],
                                    transpose=True)
                            # vb = [v_i | v_r] per key chunk (the minus of
                            # the complex product sits in the epilogue sub)
                            base = half * TBLK
                            vbv = vb[h][:, base:base + TBLK].rearrange(
                                "p (k c) -> p k c", c=P)
                            vav = va[h][:, base:base + TBLK].rearrange(
                                "p (k c) -> p k c", c=P)
                            nc.vector.tensor_copy(vbv[:, :, 0:DH],
                                                  vav[:, :, DH:P])
                            nc.vector.tensor_copy(vbv[:, :, DH:P],
                                                  vav[:, :, 0:DH])

        def emit_attention(b, qcat, kcr, kci, va, vb, o_stage):
            units = [(h, qb, kc)
                     for h in range(HPC) for qb in range(2)
                     for kc in range(KC)]
            swides = [None] * len(units)
            accs = {}

            def emit_scores(n):
                h, qb, kc = units[n]
                qs = slice(qb * TBLK, (qb + 1) * TBLK)
                ks = slice(kc * P, (kc + 1) * P)
                sw = sps.tile([P, WBLK], F32, tag="sps", name="scorew")
                nc.tensor.matmul(sw[:, 0:TBLK], kcr[h][:, ks],
                                 qcat[h][:, qs], start=True, stop=True)
                nc.tensor.matmul(sw[:, TBLK:WBLK], kci[h][:, ks],
                                 qcat[h][:, qs], start=True, stop=True)
                swides[n] = sw

            emit_scores(0)
            for n, (h, qb, kc) in enumerate(units):
                if n + 1 < len(units):
                    emit_scores(n + 1)
                uw = upool.tile([P, WBLK], BF16, tag="u", name="u")
                nc.scalar.activation(uw[:], swides[n][:], EXP)
                ur = uw[:, 0:TBLK]
                ui = uw[:, TBLK:WBLK]
                ks = slice(kc * P, (kc + 1) * P)
                first, last = kc == 0, kc == KC - 1
                if first:
                    zw = zps_pool.tile([P, WBLK], F32, tag="zw", name="zw")
                    ow = ops_pool.tile([P, WBLK], F32, tag="ow", name="ow")
                    accs[(h, qb)] = (zw, ow)
                else:
                    zw, ow = accs[(h, qb)]
                nc.tensor.matmul(zw[:, 0:TBLK], ones[:], ur,
                                 start=first, stop=last)
                nc.tensor.matmul(zw[:, TBLK:WBLK], ones[:], ui,
                                 start=first, stop=last)
                nc.tensor.matmul(ow[:, 0:TBLK], va[h][:, ks], ur,
                                 start=first, stop=last)
                nc.tensor.matmul(ow[:, TBLK:WBLK], vb[h][:, ks], ui,
                                 start=first, stop=last)
                if last:
                    # o_r = (v_r.T u_r)/Z_r - (v_i.T u_i)/Z_i : each AV term
                    # gets its OWN softmax denominator
                    qs = slice(qb * TBLK, (qb + 1) * TBLK)
                    zinvw = zpool.tile([P, WBLK], F32, tag="zinv",
                                       name="zinvw")
                    nc.vector.reciprocal_approx_fast(zinvw[:], zw[:])
                    tmpw = tmppool.tile([P, WBLK], F32, tag="tmp",
                                        name="tmpw")
                    nc.vector.tensor_mul(tmpw[:], ow[:], zinvw[:])
                    dst = slice(DH * h, DH * (h + 1))
                    nc.vector.tensor_sub(o_stage["r"][dst, qs],
                                         tmpw[0:DH, 0:TBLK],
                                         tmpw[0:DH, TBLK:WBLK])
                    nc.vector.tensor_add(o_stage["i"][dst, qs],
                                         tmpw[DH:P, 0:TBLK],
                                         tmpw[DH:P, TBLK:WBLK])

        def emit_oproj(b, o_stage):
            for half in range(2):
                hs = slice(half * TBLK, (half + 1) * TBLK)
                gt = 2 * b + half
                powide = popool.tile([P, 2 * DC * TBLK], BF16, tag="pow",
                                     name="powide")
                for mc in range(DC):
                    ms = slice(mc * P, (mc + 1) * P)
                    ow2 = sps.tile([P, WBLK], F32, tag="sps", name="ojw")
                    pr = ow2[:, 0:TBLK]
                    pi = ow2[:, TBLK:WBLK]
                    nc.tensor.matmul(pr, wot["r"][:, ms],
                                     o_stage["r"][:, hs],
                                     start=True, stop=False)
                    nc.tensor.matmul(pr, wot["in"][:, ms],
                                     o_stage["i"][:, hs],
                                     start=False, stop=True)
                    nc.tensor.matmul(pi, wot["i"][:, ms],
                                     o_stage["r"][:, hs],
                                     start=True, stop=False)
                    nc.tensor.matmul(pi, wot["r"][:, ms],
                                     o_stage["i"][:, hs],
                                     start=False, stop=True)
                    # one wide copy: [pr | pi] -> powide cols 2*mc..2*mc+2
                    nc.vector.tensor_copy(
                        powide[:, 2 * mc * TBLK:(2 * mc + 2) * TBLK], ow2[:])
                nc.sync.dma_start(po_ap[gt * P:(gt + 1) * P, :], powide[:])

        # ---- pipelined emission: one continuous tensor stream ----
        emit_xloads(0)
        stage = {}

        def new_stage(b):
            qcat = [qkpool.tile([P, S], BF16, tag=f"qcat{h}", name=f"qcat{h}")
                    for h in range(HPC)]
            kcr = [qkpool.tile([P, S], BF16, tag=f"kcr{h}", name=f"kcr{h}")
                   for h in range(HPC)]
            kci = [qkpool.tile([P, S], BF16, tag=f"kci{h}", name=f"kci{h}")
                   for h in range(HPC)]
            va = [vpool.tile([P, S], BF16, tag=f"va{h}", name=f"va{h}")
                  for h in range(HPC)]
            vb = [vpool.tile([P, S], BF16, tag=f"vb{h}", name=f"vb{h}")
                  for h in range(HPC)]
            o_stage = {p: opool.tile([P, S], BF16, tag=f"ost{p}",
                                     name=f"ost{p}")
                       for p in ("r", "i")}
            stage[b] = (qcat, kcr, kci, va, vb, o_stage)

        new_stage(0)
        emit_proj(0, *stage[0][:5])
        for b in range(B):
            if b + 1 < B:
                emit_xloads(b + 1)
            emit_attention(b, *stage[b])
            if b + 1 < B:
                new_stage(b + 1)
                emit_proj(b + 1, *stage[b + 1][:5])
            emit_oproj(b, stage[b][5])
            del stage[b]

    nc.compile()
    return nc


def _w_sbuf_layout(w_t):
    """[D, 128] weight-transpose slice -> SBUF layout [128, dc*128+o]."""
    return np.ascontiguousarray(
        w_t.reshape(DC, P, P).transpose(1, 0, 2).reshape(P, D))


def _tile_x(xT, dtype):
    """[D, B*S] -> partition-major [NT*P, DC*TBLK] (row gt*P+p, col dc*TBLK+t)."""
    t = xT.reshape(DC, P, NT, TBLK).transpose(2, 1, 0, 3)
    return np.ascontiguousarray(t.reshape(NT * P, DC * TBLK)).astype(dtype)


def _prepare_in_maps(inputs):
    bf = ml_dtypes.bfloat16
    xs = {}
    for name, t in (("queries", "q"), ("keys", "k"), ("values", "v")):
        x = np.asarray(inputs[name], dtype=np.float32)  # [B,S,D,2]
        flat = x.reshape(B * S, D, 2)
        xs[t + "r"] = _tile_x(flat[:, :, 0].T, bf)
        xs[t + "i"] = _tile_x(flat[:, :, 1].T, bf)

    scale = np.float32(1.0 / np.sqrt(DH))
    in_maps = []
    for c in range(NCORES):
        rows = slice(P * c, P * (c + 1))
        m = {}
        for t in ("q", "k", "v"):
            for part in ("r", "i"):
                m[f"x{t}_{part}"] = xs[t + part]
        for t, wr_name, wi_name in (("q", "wq_r", "wq_i"),
                                    ("k", "wk_r", "wk_i"),
                                    ("v", "wv_r", "wv_i")):
            s = scale if t == "q" else np.float32(1.0)
            wr = np.asarray(inputs[wr_name], dtype=np.float32)[rows] * s
            wi = np.asarray(inputs[wi_name], dtype=np.float32)[rows] * s
            for h in range(HPC):
                hr = slice(DH * h, DH * (h + 1))
                if t == "q":
                    wa = np.concatenate([wr[hr].T, wi[hr].T], axis=1)
                    wb = np.concatenate([-wi[hr].T, wr[hr].T], axis=1)
                elif t == "k":
                    wa = np.concatenate([wr[hr].T, -wi[hr].T], axis=1)
                    wb = np.concatenate([-wi[hr].T, -wr[hr].T], axis=1)
                else:
                    wa = np.concatenate([wr[hr].T, wi[hr].T], axis=1)
                    wb = np.concatenate([-wi[hr].T, wr[hr].T], axis=1)
                m[f"w{t}_a{h}"] = _w_sbuf_layout(wa).astype(bf)
                m[f"w{t}_b{h}"] = _w_sbuf_layout(wb).astype(bf)
        wo_r = np.asarray(inputs["wo_r"], dtype=np.float32)[:, rows]  # [D,128]
        wo_i = np.asarray(inputs["wo_i"], dtype=np.float32)[:, rows]
        m["wo_r"] = np.ascontiguousarray(wo_r.T).astype(bf)  # [128 d, 1024 m]
        m["wo_i"] = np.ascontiguousarray(wo_i.T).astype(bf)
        m["wo_in"] = np.ascontiguousarray(-wo_i.T).astype(bf)
        m["onesin"] = np.ones((P, P), dtype=bf)
        in_maps.append(m)
    return in_maps


LAST_RESULT = None


def _run(inputs, trace=False):
    global LAST_RESULT
    from concourse.bass_utils import run_bass_kernel_spmd
    if "nc" not in _CACHE:
        _CACHE["nc"] = _build()
    nc = _CACHE["nc"]
    in_maps = _prepare_in_maps(inputs)
    if trace:
        os.environ.pop("BASS_NEVER_TRACE", None)
    else:
        os.environ["BASS_NEVER_TRACE"] = "1"
    res = run_bass_kernel_spmd(nc, in_maps, core_ids=list(range(NCORES)),
                               trace=trace)
    LAST_RESULT = res
    # po rows gt*P+p, cols (2*mc+ri)*TBLK+tok
    acc = np.zeros((NT * P, 2 * DC * TBLK), np.float32)
    for c in range(NCORES):
        acc += res.results[c]["po"].astype(np.float32)

    # [NT, P, DC, 2, TBLK] -> ri, [D=DC*P? no: d=mc*P+p] , tokens
    t = acc.reshape(NT, P, DC, 2, TBLK)
    out = np.empty((B, S, D, 2), np.float32)
    for ri in range(2):
        # value at [gt, p, mc, ri, tok] = out_part[d=mc*128+p, gt*512+tok]
        comp = t[:, :, :, ri, :].transpose(2, 1, 0, 3).reshape(D, B * S)
        out[..., ri] = comp.T.reshape(B, S, D)
    return out


def kernel(**inputs):
    return _run(inputs, trace=False)


# revision 8
# speedup vs baseline: 1.2855x; 1.0493x over previous
"""ComplexMultiHeadAttention on 8 TRN2 NeuronCores (Bass/Tile).

Problem: B=4, S=1024, D_MODEL=1024, N_HEADS=16, D_HEAD=64, complex-valued
activations stored as a trailing dim of size 2 (real, imag).

    q = to_heads(complex_linear(queries, wq));  k, v likewise
    s_r + i*s_i = (q_r + i q_i)(k_r + i k_i)^T / sqrt(dh)
    a_r = softmax(s_r), a_i = softmax(s_i)      (independent softmaxes)
    o = complex_bmm(a, v);  out = complex_linear(concat_heads(o), wo)

Sharding: head-parallel. Core c owns heads {2c, 2c+1} = 128 contiguous dims
of the hidden axis. Each core computes Q/K/V projections for its 128 output
dims (weights row-sliced), runs attention for its 8 (batch, head) pairs, and
computes a partial O-projection (wo column-sliced on its 128 input dims)
over all 1024 output dims. The host sums the 8 partial outputs — no
on-device collectives.

Performance notes (TRN2):
  - The PE p-state ramps: any idle gap drops the clock to 1.2 GHz for the
    next ~3us. The whole program is therefore emitted as one continuous
    tensor stream: attention(b) -> proj(b+1) -> oproj(b) -> attention(b+1),
    with the attention inner loop software-pipelined (scores for chunk n+1
    are emitted before the exp-gated consumers of chunk n).
  - DMA cost is per-partition-LINE (~5.6ns/line regardless of 1KB vs 2KB),
    so inputs use a partition-major DRAM layout ([gt*128+p, dc*512+tok])
    giving 8KB contiguous lines: one 1MB DMA per (tensor, part, gt) instead
    of eight 128KB DMAs with 1KB lines. Outputs are batched the same way.
  - V is transposed to token-major via DMA-transpose on the scalar HWDGE
    queue (not the PE), with V projected FIRST so the transposes overlap
    the q/k projection matmuls.
  - scores r|i land in the two banks of one wide [128,1024] PSUM tile; a
    single wide EXP covers both (scalar engine issue rate ~1.15us/unit vs
    1.28us of tensor work per unit -> attention stays tensor-paced).
  - All matmuls bf16 (f32 PSUM accumulation); softmax over k (=partitions)
    skips max-subtraction (scores are O(1) by construction) and takes Z
    with a ones[128,128] matmul; 1/Z is one wide reciprocal + one wide mul.
PSUM budget (8 banks): wide proj/score pool 2x2 + wide Z 2 + wide AV 2.
"""

import os
import numpy as np
import ml_dtypes
from contextlib import ExitStack

import concourse.bass as bass
import concourse.tile as tile
from concourse import bacc, mybir

F32 = mybir.dt.float32
BF16 = mybir.dt.bfloat16
EXP = mybir.ActivationFunctionType.Exp

B, S, D, H, DH = 4, 1024, 1024, 16, 64
NCORES = 8
P = 128            # partitions / chunk size
TBLK = 512         # token block (matmul free dim)
WBLK = 2 * TBLK    # wide psum tile (2 banks)
DC = D // P        # 8 d-chunks
KC = S // P        # 8 key chunks per batch
HPC = H // NCORES  # 2 heads per core
NT = (B * S) // TBLK

_CACHE = {}


def _build():
    nc = bacc.Bacc("TRN2", target_bir_lowering=False, debug=False,
                   num_devices=NCORES)

    # partition-major tiled layout: row gt*128+p, col dc*512+tok
    x_ap = {}
    for t in ("q", "k", "v"):
        for part in ("r", "i"):
            x_ap[t + part] = nc.dram_tensor(
                f"x{t}_{part}", [NT * P, DC * TBLK],
                BF16, kind="ExternalInput").ap()
    w_ap = {}
    for t in ("q", "k", "v"):
        for h in range(HPC):
            for suf in ("a", "b"):
                w_ap[f"{t}{suf}{h}"] = nc.dram_tensor(
                    f"w{t}_{suf}{h}", [P, D], BF16, kind="ExternalInput").ap()
    wo_ap = {}
    for suf in ("r", "i", "in"):
        wo_ap[suf] = nc.dram_tensor(
            f"wo_{suf}", [P, D], BF16, kind="ExternalInput").ap()
    ones_ap = nc.dram_tensor("onesin", [P, P], BF16, kind="ExternalInput").ap()
    # output: row gt*128+p, col (2*mc+ri)*512+tok  (r/i interleaved per mc)
    po_ap = nc.dram_tensor("po", [NT * P, 2 * DC * TBLK], BF16,
                           kind="ExternalOutput").ap()

    with tile.TileContext(nc) as tc, ExitStack() as ctx:
        wpool = ctx.enter_context(tc.tile_pool(name="w", bufs=1))
        xpool = ctx.enter_context(tc.tile_pool(name="x", bufs=7))
        qkpool = ctx.enter_context(tc.tile_pool(name="qk", bufs=2))
        vpool = ctx.enter_context(tc.tile_pool(name="v", bufs=2))
        opool = ctx.enter_context(tc.tile_pool(name="ost", bufs=2))
        upool = ctx.enter_context(tc.tile_pool(name="u", bufs=3))
        zpool = ctx.enter_context(tc.tile_pool(name="z", bufs=2))
        tmppool = ctx.enter_context(tc.tile_pool(name="tmp", bufs=2))
        popool = ctx.enter_context(tc.tile_pool(name="po", bufs=2))
        vstpool = ctx.enter_context(tc.tile_pool(name="vst", bufs=4))
        # PSUM: 8 banks: wide 2-bank [128,1024] proj/score/o-proj pool x2,
        # plus FOUR separate 1-bank accumulators (zr, zi, oa, ob) so each
        # frees as soon as its own epilogue read completes (the next octet's
        # kc0 matmuls reuse them ~1.4us after the previous octet ends).
        sps = ctx.enter_context(tc.tile_pool(name="sp", bufs=2, space="PSUM"))
        zps_pool = ctx.enter_context(tc.tile_pool(name="zp", bufs=1,
                                                  space="PSUM"))
        ops_pool = ctx.enter_context(tc.tile_pool(name="op", bufs=1,
                                                  space="PSUM"))

        # weights on the scalar HWDGE queue (idle at start; sync queue
        # starts on the batch-0 x loads in parallel)
        wt = {}
        for key, ap in list(w_ap.items()):
            wt[key] = wpool.tile([P, D], BF16, tag=f"w_{key}", name=f"w_{key}")
            nc.scalar.dma_start(wt[key][:], ap[:])
        wot = {}
        for suf, ap in wo_ap.items():
            wot[suf] = wpool.tile([P, D], BF16, tag=f"wo_{suf}",
                                  name=f"wo_{suf}")
            nc.scalar.dma_start(wot[suf][:], ap[:])
        ones = wpool.tile([P, P], BF16, tag="ones", name="ones")
        nc.scalar.dma_start(ones[:], ones_ap[:])

        xtiles = {}

        def emit_xloads(b):
            # v first (projection order is v,q,k)
            for t in ("v", "q", "k"):
                for part in ("r", "i"):
                    for half in range(2):
                        gt = 2 * b + half
                        xt = xpool.tile([P, DC * TBLK], BF16, tag="xt",
                                        name="xt")
                        nc.sync.dma_start(
                            xt[:], x_ap[t + part][gt * P:(gt + 1) * P, :])
                        xtiles[(b, t, part, half)] = xt

        def emit_proj(b, qcat, kcr, kci, va, vb):
            # v FIRST so its DMA-transposes (scalar queue) overlap the q/k
            # projection matmuls
            for t in ("v", "q", "k"):
                wA = (wt[t + "a0"], wt[t + "a1"])
                wB = (wt[t + "b0"], wt[t + "b1"])
                for half in range(2):
                    xr = xtiles.pop((b, t, "r", half))
                    xi = xtiles.pop((b, t, "i", half))
                    pwide = sps.tile([P, WBLK], F32, tag="sps", name="projw")
                    psr = pwide[:, 0:TBLK]
                    psi = pwide[:, TBLK:WBLK]
                    for dc in range(DC):
                        ws = slice(dc * P, (dc + 1) * P)
                        xs_ = slice(dc * TBLK, (dc + 1) * TBLK)
                        nc.tensor.matmul(psr, wA[0][:, ws], xr[:, xs_],
                                         start=(dc == 0), stop=False)
                        nc.tensor.matmul(psi, wA[1][:, ws], xr[:, xs_],
                                         start=(dc == 0), stop=False)
                    for dc in range(DC):
                        ws = slice(dc * P, (dc + 1) * P)
                        xs_ = slice(dc * TBLK, (dc + 1) * TBLK)
                        nc.tensor.matmul(psr, wB[0][:, ws], xi[:, xs_],
                                         start=False, stop=(dc == DC - 1))
                        nc.tensor.matmul(psi, wB[1][:, ws], xi[:, xs_],
                                         start=False, stop=(dc == DC - 1))
                    hs = slice(half * TBLK, (half + 1) * TBLK)
                    if t == "q":
                        # psX = [q_r(h); q_i(h)] = Qcat directly
                        for h, psx in ((0, psr), (1, psi)):
                            nc.vector.tensor_copy(qcat[h][:, hs], psx)
                    elif t == "k":
                        # psX = [k_r(h); -k_i(h)] = Kcat_r directly;
                        # Kcat_i = [k_i; k_r] via one negate + one copy
                        for h, psx in ((0, psr), (1, psi)):
                            nc.vector.tensor_copy(kcr[h][:, hs], psx)
                            nc.vector.tensor_scalar_mul(
                                kci[h][0:DH, hs], psx[DH:P, :], -1.0)
                            nc.vector.tensor_copy(kci[h][DH:P, hs],
                                                  psx[0:DH, :])
                    else:
                        # psr = [v_r(h0); v_i(h0)], psi = [v_r(h1); v_i(h1)]
                        # DMA-transpose (scalar HWDGE) to token-major packs
                        for h, psx in ((0, psr), (1, psi)):
                            vst = vstpool.tile([P, TBLK], BF16, tag="vst",
                                               name="vst")
                            nc.vector.tensor_copy(vst[:], psx)
                            for blk in range(4):
                                kcg = half * 4 + blk
                                nc.scalar.dma_start(
                                    va[h][:, kcg * P:(kcg + 1) * P],
                                    vst[:, blk * P:(blk + 1) * P],
                                    transpose=True)
                            # vb = [v_i | v_r] per key chunk (the minus of
                            # the complex product sits in the epilogue sub)
                            base = half * TBLK
                            vbv = vb[h][:, base:base + TBLK].rearrange(
                                "p (k c) -> p k c", c=P)
                            vav = va[h][:, base:base + TBLK].rearrange(
                                "p (k c) -> p k c", c=P)
                            nc.vector.tensor_copy(vbv[:, :, 0:DH],
                                                  vav[:, :, DH:P])
                            nc.vector.tensor_copy(vbv[:, :, DH:P],
                                                  vav[:, :, 0:DH])

        def emit_attention(b, qcat, kcr, kci, va, vb, o_stage):
            units = [(h, qb, kc)
                     for h in range(HPC) for qb in range(2)
                     for kc in range(KC)]
            swides = [None] * len(units)
            accs = {}

            def emit_scores(n):
                h, qb, kc = units[n]
                qs = slice(qb * TBLK, (qb + 1) * TBLK)
                ks = slice(kc * P, (kc + 1) * P)
                sw = sps.tile([P, WBLK], F32, tag="sps", name="scorew")
                nc.tensor.matmul(sw[:, 0:TBLK], kcr[h][:, ks],
                                 qcat[h][:, qs], start=True, stop=True)
                nc.tensor.matmul(sw[:, TBLK:WBLK], kci[h][:, ks],
                                 qcat[h][:, qs], start=True, stop=True)
                swides[n] = sw

            emit_scores(0)
            for n, (h, qb, kc) in enumerate(units):
                if n + 1 < len(units):
                    emit_scores(n + 1)
                uw = upool.tile([P, WBLK], BF16, tag="u", name="u")
                nc.scalar.activation(uw[:], swides[n][:], EXP)
                ur = uw[:, 0:TBLK]
                ui = uw[:, TBLK:WBLK]
                ks = slice(kc * P, (kc + 1) * P)
                first, last = kc == 0, kc == KC - 1
                if first:
                    zr = zps_pool.tile([P, TBLK], F32, tag="zr", name="zr")
                    zi = zps_pool.tile([P, TBLK], F32, tag="zi", name="zi")
                    oa = ops_pool.tile([P, TBLK], F32, tag="oa", name="oa")
                    ob = ops_pool.tile([P, TBLK], F32, tag="ob", name="ob")
                    accs[(h, qb)] = (zr, zi, oa, ob)
                else:
                    zr, zi, oa, ob = accs[(h, qb)]
                nc.tensor.matmul(zr[:], ones[:], ur,
                                 start=first, stop=last)
                nc.tensor.matmul(zi[:], ones[:], ui,
                                 start=first, stop=last)
                nc.tensor.matmul(oa[:], va[h][:, ks], ur,
                                 start=first, stop=last)
                nc.tensor.matmul(ob[:], vb[h][:, ks], ui,
                                 start=first, stop=last)
                if last:
                    # o_r = (v_r.T u_r)/Z_r - (v_i.T u_i)/Z_i : each AV term
                    # gets its OWN softmax denominator.  Ordered so each
                    # accumulator bank frees as early as possible.
                    qs = slice(qb * TBLK, (qb + 1) * TBLK)
                    zinv_r = zpool.tile([P, TBLK], F32, tag="zinvr",
                                        name="zinv_r")
                    nc.vector.reciprocal_approx_fast(zinv_r[:], zr[:])
                    zinv_i = zpool.tile([P, TBLK], F32, tag="zinvi",
                                        name="zinv_i")
                    nc.vector.reciprocal_approx_fast(zinv_i[:], zi[:])
                    tmpa = tmppool.tile([P, TBLK], F32, tag="tmpa",
                                        name="tmpa")
                    nc.vector.tensor_mul(tmpa[:], oa[:], zinv_r[:])
                    tmpb = tmppool.tile([P, TBLK], F32, tag="tmpb",
                                        name="tmpb")
                    nc.vector.tensor_mul(tmpb[:], ob[:], zinv_i[:])
                    dst = slice(DH * h, DH * (h + 1))
                    nc.vector.tensor_sub(o_stage["r"][dst, qs],
                                         tmpa[0:DH, :], tmpb[0:DH, :])
                    nc.vector.tensor_add(o_stage["i"][dst, qs],
                                         tmpa[DH:P, :], tmpb[DH:P, :])

        def emit_oproj(b, o_stage):
            for half in range(2):
                hs = slice(half * TBLK, (half + 1) * TBLK)
                gt = 2 * b + half
                powide = popool.tile([P, 2 * DC * TBLK], BF16, tag="pow",
                                     name="powide")
                for mc in range(DC):
                    ms = slice(mc * P, (mc + 1) * P)
                    ow2 = sps.tile([P, WBLK], F32, tag="sps", name="ojw")
                    pr = ow2[:, 0:TBLK]
                    pi = ow2[:, TBLK:WBLK]
                    nc.tensor.matmul(pr, wot["r"][:, ms],
                                     o_stage["r"][:, hs],
                                     start=True, stop=False)
                    nc.tensor.matmul(pr, wot["in"][:, ms],
                                     o_stage["i"][:, hs],
                                     start=False, stop=True)
                    nc.tensor.matmul(pi, wot["i"][:, ms],
                                     o_stage["r"][:, hs],
                                     start=True, stop=False)
                    nc.tensor.matmul(pi, wot["r"][:, ms],
                                     o_stage["i"][:, hs],
                                     start=False, stop=True)
                    # one wide copy: [pr | pi] -> powide cols 2*mc..2*mc+2
                    nc.vector.tensor_copy(
                        powide[:, 2 * mc * TBLK:(2 * mc + 2) * TBLK], ow2[:])
                # store on the gpsimd SWDGE queue: the sync queue must stay
                # dedicated to input prefetch (a store blocking the sync
                # FIFO head starves the next projection phase)
                nc.gpsimd.dma_start(po_ap[gt * P:(gt + 1) * P, :], powide[:])

        # ---- pipelined emission: one continuous tensor stream ----
        emit_xloads(0)
        stage = {}

        def new_stage(b):
            qcat = [qkpool.tile([P, S], BF16, tag=f"qcat{h}", name=f"qcat{h}")
                    for h in range(HPC)]
            kcr = [qkpool.tile([P, S], BF16, tag=f"kcr{h}", name=f"kcr{h}")
                   for h in range(HPC)]
            kci = [qkpool.tile([P, S], BF16, tag=f"kci{h}", name=f"kci{h}")
                   for h in range(HPC)]
            va = [vpool.tile([P, S], BF16, tag=f"va{h}", name=f"va{h}")
                  for h in range(HPC)]
            vb = [vpool.tile([P, S], BF16, tag=f"vb{h}", name=f"vb{h}")
                  for h in range(HPC)]
            o_stage = {p: opool.tile([P, S], BF16, tag=f"ost{p}",
                                     name=f"ost{p}")
                       for p in ("r", "i")}
            stage[b] = (qcat, kcr, kci, va, vb, o_stage)

        new_stage(0)
        emit_proj(0, *stage[0][:5])
        for b in range(B):
            if b + 1 < B:
                emit_xloads(b + 1)
            emit_attention(b, *stage[b])
            if b + 1 < B:
                new_stage(b + 1)
                emit_proj(b + 1, *stage[b + 1][:5])
            emit_oproj(b, stage[b][5])
            del stage[b]

    nc.compile()
    return nc


def _w_sbuf_layout(w_t):
    """[D, 128] weight-transpose slice -> SBUF layout [128, dc*128+o]."""
    return np.ascontiguousarray(
        w_t.reshape(DC, P, P).transpose(1, 0, 2).reshape(P, D))


def _tile_x(xT, dtype):
    """[D, B*S] -> partition-major [NT*P, DC*TBLK] (row gt*P+p, col dc*TBLK+t)."""
    t = xT.reshape(DC, P, NT, TBLK).transpose(2, 1, 0, 3)
    return np.ascontiguousarray(t.reshape(NT * P, DC * TBLK)).astype(dtype)


def _prepare_in_maps(inputs):
    bf = ml_dtypes.bfloat16
    xs = {}
    for name, t in (("queries", "q"), ("keys", "k"), ("values", "v")):
        x = np.asarray(inputs[name], dtype=np.float32)  # [B,S,D,2]
        flat = x.reshape(B * S, D, 2)
        xs[t + "r"] = _tile_x(flat[:, :, 0].T, bf)
        xs[t + "i"] = _tile_x(flat[:, :, 1].T, bf)

    scale = np.float32(1.0 / np.sqrt(DH))
    in_maps = []
    for c in range(NCORES):
        rows = slice(P * c, P * (c + 1))
        m = {}
        for t in ("q", "k", "v"):
            for part in ("r", "i"):
                m[f"x{t}_{part}"] = xs[t + part]
        for t, wr_name, wi_name in (("q", "wq_r", "wq_i"),
                                    ("k", "wk_r", "wk_i"),
                                    ("v", "wv_r", "wv_i")):
            s = scale if t == "q" else np.float32(1.0)
            wr = np.asarray(inputs[wr_name], dtype=np.float32)[rows] * s
            wi = np.asarray(inputs[wi_name], dtype=np.float32)[rows] * s
            for h in range(HPC):
                hr = slice(DH * h, DH * (h + 1))
                if t == "q":
                    wa = np.concatenate([wr[hr].T, wi[hr].T], axis=1)
                    wb = np.concatenate([-wi[hr].T, wr[hr].T], axis=1)
                elif t == "k":
                    wa = np.concatenate([wr[hr].T, -wi[hr].T], axis=1)
                    wb = np.concatenate([-wi[hr].T, -wr[hr].T], axis=1)
                else:
                    wa = np.concatenate([wr[hr].T, wi[hr].T], axis=1)
                    wb = np.concatenate([-wi[hr].T, wr[hr].T], axis=1)
                m[f"w{t}_a{h}"] = _w_sbuf_layout(wa).astype(bf)
                m[f"w{t}_b{h}"] = _w_sbuf_layout(wb).astype(bf)
        wo_r = np.asarray(inputs["wo_r"], dtype=np.float32)[:, rows]  # [D,128]
        wo_i = np.asarray(inputs["wo_i"], dtype=np.float32)[:, rows]
        m["wo_r"] = np.ascontiguousarray(wo_r.T).astype(bf)  # [128 d, 1024 m]
        m["wo_i"] = np.ascontiguousarray(wo_i.T).astype(bf)
        m["wo_in"] = np.ascontiguousarray(-wo_i.T).astype(bf)
        m["onesin"] = np.ones((P, P), dtype=bf)
        in_maps.append(m)
    return in_maps


LAST_RESULT = None


def _run(inputs, trace=False):
    global LAST_RESULT
    from concourse.bass_utils import run_bass_kernel_spmd
    if "nc" not in _CACHE:
        _CACHE["nc"] = _build()
    nc = _CACHE["nc"]
    in_maps = _prepare_in_maps(inputs)
    if trace:
        os.environ.pop("BASS_NEVER_TRACE", None)
    else:
        os.environ["BASS_NEVER_TRACE"] = "1"
    res = run_bass_kernel_spmd(nc, in_maps, core_ids=list(range(NCORES)),
                               trace=trace)
    LAST_RESULT = res
    # po rows gt*P+p, cols (2*mc+ri)*TBLK+tok
    acc = np.zeros((NT * P, 2 * DC * TBLK), np.float32)
    for c in range(NCORES):
        acc += res.results[c]["po"].astype(np.float32)

    # [NT, P, DC, 2, TBLK] -> ri, [D=DC*P? no: d=mc*P+p] , tokens
    t = acc.reshape(NT, P, DC, 2, TBLK)
    out = np.empty((B, S, D, 2), np.float32)
    for ri in range(2):
        # value at [gt, p, mc, ri, tok] = out_part[d=mc*128+p, gt*512+tok]
        comp = t[:, :, :, ri, :].transpose(2, 1, 0, 3).reshape(D, B * S)
        out[..., ri] = comp.T.reshape(B, S, D)
    return out


def kernel(**inputs):
    return _run(inputs, trace=False)
